# revision 10
# baseline (speedup 1.0000x reference)
"""DetectionLoss Trainium2 Bass kernel (v3: sparse-only, tuned critical path).

Data-parallel over batch: 2 images per core x 8 cores; host sums per-box
partials (npos is a global normalizer, so per-core normalization is
impossible anyway - the sharding hint's "per-shard sums + counts").

The reference only needs logsumexp at positive cells, so the host repacks
(pure relayout) obj/reg/cls into per-cell records [B*HW, 64] (obj, reg0..3,
cls0..29, pad to a 256B row) and the device fetches the <=384 needed rows
with three indirect DMAs (~1.2us each on GpSimd; a single dma_gather was
probed at ~11us end-to-end and rejected). The dense phase is just the obj
softplus over 16.8k logits, padded with -88 so pad cells add exactly 0.

v3 vs v2 (27.9us):
  - one hot input DMA [128,20] (boxes+labels-as-f32+key consts) instead of
    five small ones: the sync sequencer's DIRECT2D descriptor gen is ~700ns
    per DMA and serialized; cold consts ride one DMA on the scalar queue.
  - act-table discipline enforced with add_dep_helper: the tile scheduler
    had hoisted the obj Lns between the Exp ops, costing two extra 1.28us
    ACT_TABLE_LOADs on the critical path.
  - CE exp per scale with accum_out -> sum-exp lands on the scalar engine
    directly (no cross-engine DVE reduce before the ln).
  - output is the raw per-box stack [128,18]; host sums partitions (it
    already sums cores). Kills the PE transpose + DVE reduce + tiny-DMA
    tail.
  - fused dtype converts in the key chain and abs via abs_max (probed).
"""

import numpy as np

import concourse.bass as bass
import concourse.tile as tile
from concourse import bacc, mybir
from concourse.bass_utils import run_bass_kernel_spmd
from concourse.tile_rust import add_dep_helper

F32 = mybir.dt.float32
I32 = mybir.dt.int32
AF = mybir.ActivationFunctionType
OP = mybir.AluOpType
AX = mybir.AxisListType

# --- probed op-pattern switches ---
FUSE_CONVERTS = True   # ts/tt with dtype-converting output
ABS_MAX = False        # abs_max is not a valid DVE tensor_scalar op (probed)
USE_SOFTPLUS = False   # AF.Softplus table exists on gen3
GPSIMD_VAL = False     # Pool engine fails the ISA check on broadcast/strided tt

B_TOT = 16
N_CORES = 8
B_SH = B_TOT // N_CORES
NBOX = 64
NP = B_SH * NBOX  # 128 partitions: (image, box)
C = 30
SCALES = [(80, 80), (40, 40), (20, 20)]
HWS = [h * w for h, w in SCALES]
REC_BASE = [0, B_SH * HWS[0], B_SH * (HWS[0] + HWS[1])]  # 0, 12800, 16000
NREC = B_SH * sum(HWS)  # 16800
RECW = 64  # padded record row (256B)
BIG = 1.0e9
OBJ_COLS = [100, 25, 7]  # 12800=128x100, 3200=128x25, 800 -> 128x7 padded
OBJ_PAD = -88.0  # softplus(-88) == 0 exactly in f32

CLS_W, REG_W, OBJ_W = 1.0, 5.0, 1.0
NPART = 18  # per scale s, cols 6s + [lse, clsval, sl1, obj, softplus, npos]

# hot tile columns: 0:4 box cxcywh, 4 label(f32), 5:11 xy mults, 11:17 clips,
# 17:20 record-row offsets
HOTW = 20
# cold tile columns: 0:30 iota, 30:158 identity, 158:286 utri, 286:418 objd
COLDW = 30 + 256 + sum(OBJ_COLS)


def _host_consts():
    p = np.arange(128)
    bvec = (p >= NBOX).astype(np.float32)
    kc = np.zeros((128, 15), np.float32)
    for s, (h, w) in enumerate(SCALES):
        kc[:, 0 + s] = w
        kc[:, 3 + s] = h
        kc[:, 6 + s] = w - 1
        kc[:, 9 + s] = h - 1
        kc[:, 12 + s] = REC_BASE[s] + bvec * (h * w)
    iota = np.tile(np.arange(C, dtype=np.float32), (128, 1))
    ident = np.eye(128, dtype=np.float32)
    utri = np.triu(np.ones((128, 128), np.float32), 1)
    return kc, iota, ident, utri


def emit(tc: tile.TileContext, out_ap, ins):
    nc = tc.nc

    pool = tc.alloc_tile_pool(name="sb", bufs=1)
    kmps = tc.alloc_tile_pool(name="kmps", bufs=1, space="PSUM")
    lbps = tc.alloc_tile_pool(name="lbps", bufs=1, space="PSUM")

    # ---- two input DMAs: hot (key chain) on sync, cold on scalar ----
    hot = pool.tile([128, HOTW], F32, tag="hot")
    nc.sync.dma_start(out=hot[:], in_=ins["hot"])
    cold = pool.tile([128, COLDW], F32, tag="cold")
    nc.scalar.dma_start(out=cold[:], in_=ins["cold"])

    iott = cold[:, 0:30]
    ident = cold[:, 30:158]
    utri = cold[:, 158:286]
    objd = cold[:, 286 : 286 + sum(OBJ_COLS)]
    labf = hot[:, 4:5]
    kxy = hot[:, 5:11].rearrange("p (c s) -> p c s", c=2)
    kclip = hot[:, 11:17].rearrange("p (c s) -> p c s", c=2)

    # ---- keys: floor(x) = round-to-nearest(x - 0.5) via i32 convert ----
    gr = pool.tile([NP, 2, 3], F32, tag="gr")
    nc.vector.tensor_tensor(
        out=gr[:], in0=hot[:, 0:2, None].to_broadcast([NP, 2, 3]), in1=kxy, op=OP.mult
    )
    gf = pool.tile([NP, 2, 3], F32, tag="gf")
    if FUSE_CONVERTS:
        gi = pool.tile([NP, 2, 3], I32, tag="gi")
        nc.vector.tensor_scalar(out=gi[:], in0=gr[:], scalar1=-0.5, scalar2=None, op0=OP.add)
        nc.vector.tensor_tensor(out=gf[:], in0=gi[:], in1=kclip, op=OP.min)
    else:
        nc.vector.tensor_scalar(out=gr[:], in0=gr[:], scalar1=-0.5, scalar2=None, op0=OP.add)
        gi = pool.tile([NP, 2, 3], I32, tag="gi")
        nc.vector.tensor_copy(out=gi[:], in_=gr[:])
        nc.vector.tensor_copy(out=gf[:], in_=gi[:])
        nc.vector.tensor_tensor(out=gf[:], in0=gf[:], in1=kclip, op=OP.min)

    kt = pool.tile([NP, 3], F32, tag="kt")
    nc.vector.tensor_tensor(out=kt[:], in0=gf[:, 1, :], in1=hot[:, 5:8], op=OP.mult)
    nc.vector.tensor_add(kt[:], kt[:], gf[:, 0, :])
    keyi = pool.tile([NP, 3], I32, tag="keyi")
    keyf = pool.tile([NP, 3], F32, tag="keyf")
    if FUSE_CONVERTS:
        nc.vector.tensor_tensor(out=keyi[:], in0=kt[:], in1=hot[:, 17:20], op=OP.add)
        nc.vector.tensor_copy(out=keyf[:], in_=keyi[:])
    else:
        nc.vector.tensor_tensor(out=keyf[:], in0=kt[:], in1=hot[:, 17:20], op=OP.add)
        nc.vector.tensor_copy(out=keyi[:], in_=keyf[:])

    # ---- record gathers: one indirect DMA per scale (GpSimd-serialized) ----
    rec = pool.tile([NP, 3 * RECW], F32, tag="rec")
    recv = rec[:].rearrange("p (s r) -> p s r", r=RECW)
    gathers = []
    for s in range(3):
        g = nc.gpsimd.indirect_dma_start(
            out=recv[:, s, :],
            out_offset=None,
            in_=ins["rec"],
            in_offset=bass.IndirectOffsetOnAxis(ap=keyi[:, s : s + 1], axis=0),
        )
        gathers.append(g)

    # ---- masks (PE+DVE), fully overlapped with the gather flight ----
    labmat = lbps.tile([128, 128], F32, tag="labmat")
    nc.tensor.transpose(
        out=labmat[:], in_=labf.to_broadcast([128, 128]), identity=ident
    )
    kmat = kmps.tile([128, 3 * 128], F32, tag="kmat")
    kmv = kmat[:].rearrange("p (s q) -> p s q", q=128)
    for s in range(3):
        nc.tensor.transpose(
            out=kmv[:, s, :],
            in_=keyf[:, s : s + 1].to_broadcast([128, 128]),
            identity=ident,
        )

    eqm = pool.tile([128, 3, 128], F32, tag="eqm")
    nc.vector.tensor_tensor(
        out=eqm[:], in0=kmv, in1=keyf[:, :, None].to_broadcast([128, 3, 128]),
        op=OP.is_equal,
    )
    lose = pool.tile([128, 3, 128], F32, tag="lose")
    nc.vector.tensor_tensor(
        out=lose[:], in0=eqm[:], in1=utri[:, None, :].to_broadcast([128, 3, 128]),
        op=OP.mult,
    )
    losev = pool.tile([NP, 3], F32, tag="losev")
    nc.vector.tensor_reduce(out=losev[:], in_=lose[:], axis=AX.X, op=OP.max)
    cnd = pool.tile([128, 3, 128], F32, tag="cnd")
    nc.vector.tensor_scalar(
        out=cnd[:], in0=eqm[:], scalar1=-BIG, scalar2=BIG, op0=OP.mult, op1=OP.add
    )
    nc.vector.tensor_tensor(
        out=cnd[:], in0=cnd[:], in1=labmat[:, None, :].to_broadcast([128, 3, 128]),
        op=OP.add,
    )
    minlab3 = pool.tile([NP, 3], F32, tag="minlab3")
    nc.vector.tensor_reduce(out=minlab3[:], in_=cnd[:], axis=AX.X, op=OP.min)

    # one-hot(minlab) is gather-independent: keep it ahead of the
    # rec-gated ops in the in-order DVE queue
    eqc = pool.tile([NP, 3, C], F32, tag="eqc")
    nc.vector.tensor_tensor(
        out=eqc[:], in0=iott[:, None, :].to_broadcast([NP, 3, C]),
        in1=minlab3[:, :, None].to_broadcast([NP, 3, C]), op=OP.is_equal,
    )

    stack = pool.tile([128, NPART], F32, tag="stack")
    stv = stack[:].rearrange("p (s j) -> p s j", j=6)
    win3 = stv[:, :, 5]  # winner mask doubles as the npos partial
    nc.vector.tensor_scalar(
        out=win3, in0=losev[:], scalar1=-1.0, scalar2=1.0, op0=OP.mult, op1=OP.add
    )

    # ---- scalar engine: obj softplus + CE, act-table order pinned ----
    # table sequence must be [SP], Exp..., Ln: one post-gather load only.
    se3 = pool.tile([NP, 3], F32, tag="se3")
    expc = pool.tile([NP, 3, C], F32, tag="expc")
    dep_chain = []

    def scalar_op(inst):
        if dep_chain:
            add_dep_helper(inst.ins, dep_chain[-1].ins, reason="act table order")
        dep_chain.append(inst)
        return inst

    dmy = pool.tile([128, 1], F32, tag="dmy")
    if USE_SOFTPLUS:
        scalar_op(nc.scalar.activation(out=dmy[:], in_=hot[:, 0:1], func=AF.Softplus, scale=0.0))
        c0 = 0
        for s in range(3):
            objl = pool.tile([128, OBJ_COLS[s]], F32, tag=f"objl{s}")
            scalar_op(nc.scalar.activation(
                out=objl[:], in_=objd[:, c0 : c0 + OBJ_COLS[s]], func=AF.Softplus,
                accum_out=stack[:, 6 * s + 4 : 6 * s + 5],
            ))
            c0 += OBJ_COLS[s]
        scalar_op(nc.scalar.activation(out=dmy[:], in_=hot[:, 0:1], func=AF.Exp, scale=0.0))
        for s in range(3):
            scalar_op(nc.scalar.activation(
                out=expc[:, s, :], in_=recv[:, s, 5 : 5 + C], func=AF.Exp,
                accum_out=se3[:, s : s + 1],
            ))
        lse3 = pool.tile([NP, 3], F32, tag="lse3")
        scalar_op(nc.scalar.activation(out=lse3[:], in_=se3[:], func=AF.Ln))
    else:
        # obj softplus = Exp then Ln(x+1), both entirely pre-gather: its Ln
        # table load hides in the gather flight. A dummy Exp then re-arms
        # the Exp table before the gathered rows land, so only the lse Ln's
        # table load sits on the post-gather critical path.
        obje = pool.tile([128, sum(OBJ_COLS)], F32, tag="obje")
        scalar_op(nc.scalar.activation(out=obje[:], in_=objd, func=AF.Exp))
        c0 = 0
        for s in range(3):
            objl = pool.tile([128, OBJ_COLS[s]], F32, tag=f"objl{s}")
            scalar_op(nc.scalar.activation(
                out=objl[:], in_=obje[:, c0 : c0 + OBJ_COLS[s]], func=AF.Ln, bias=1.0,
                accum_out=stack[:, 6 * s + 4 : 6 * s + 5],
            ))
            c0 += OBJ_COLS[s]
        scalar_op(nc.scalar.activation(out=dmy[:], in_=hot[:, 0:1], func=AF.Exp, scale=0.0))
        for s in range(3):
            scalar_op(nc.scalar.activation(
                out=expc[:, s, :], in_=recv[:, s, 5 : 5 + C], func=AF.Exp,
                accum_out=se3[:, s : s + 1],
            ))
        lse3 = pool.tile([NP, 3], F32, tag="lse3")
        scalar_op(nc.scalar.activation(out=lse3[:], in_=se3[:], func=AF.Ln))

    # ---- smooth-L1 over gathered reg records (DVE) ----
    d3 = pool.tile([NP, 3, 4], F32, tag="d3")
    nc.vector.tensor_tensor(
        out=d3[:], in0=recv[:, :, 1:5], in1=hot[:, None, 0:4].to_broadcast([NP, 3, 4]),
        op=OP.subtract,
    )
    a3 = pool.tile([NP, 3, 4], F32, tag="a3")
    if ABS_MAX:
        nc.vector.tensor_scalar(out=a3[:], in0=d3[:], scalar1=0.0, scalar2=None, op0=OP.abs_max)
    else:
        nc.vector.tensor_scalar(out=a3[:], in0=d3[:], scalar1=-1.0, scalar2=None, op0=OP.mult)
        nc.vector.tensor_tensor(out=a3[:], in0=a3[:], in1=d3[:], op=OP.max)
    q3 = pool.tile([NP, 3, 4], F32, tag="q3")
    nc.vector.tensor_scalar_min(q3[:], a3[:], 1.0)
    h3 = pool.tile([NP, 3, 4], F32, tag="h3")
    nc.vector.tensor_scalar(out=h3[:], in0=q3[:], scalar1=-0.5, scalar2=None, op0=OP.mult)
    nc.vector.tensor_add(h3[:], h3[:], a3[:])
    nc.vector.tensor_mul(h3[:], h3[:], q3[:])
    sl13 = pool.tile([NP, 3], F32, tag="sl13")
    nc.vector.tensor_reduce(out=sl13[:], in_=h3[:], axis=AX.X, op=OP.add)
    nc.vector.tensor_scalar(
        out=sl13[:], in0=sl13[:], scalar1=0.25, scalar2=10.0, op0=OP.mult, op1=OP.min
    )

    # ---- cls target logit: one-hot dot gathered row ----
    nc.vector.tensor_mul(eqc[:], eqc[:], recv[:, :, 5 : 5 + C])
    val3 = pool.tile([NP, 3], F32, tag="val3")
    nc.vector.tensor_reduce(out=val3[:], in_=eqc[:], axis=AX.X, op=OP.add)

    # ---- masked per-box partials; host sums partitions ----
    nc.vector.tensor_mul(stv[:, :, 2], sl13[:], win3[:])
    nc.vector.tensor_mul(stv[:, :, 3], recv[:, :, 0], win3[:])
    nc.vector.tensor_mul(stv[:, :, 1], val3[:], win3[:])
    nc.vector.tensor_mul(stv[:, :, 0], lse3[:], win3[:])

    nc.sync.dma_start(out=out_ap, in_=stack[:])

    lbps.release()
    kmps.release()
    pool.release()


# ---------------------------------------------------------------------------
# host side
# ---------------------------------------------------------------------------

_CACHE = {}


def _build():
    if "nc" in _CACHE:
        return _CACHE["nc"]
    nc = bacc.Bacc(
        "TRN2",
        target_bir_lowering=False,
        debug=False,
        enable_asserts=False,
        num_devices=N_CORES,
    )
    ins = {
        "rec": nc.dram_tensor("rec", (NREC, RECW), F32, kind="ExternalInput").ap(),
        "hot": nc.dram_tensor("hot", (128, HOTW), F32, kind="ExternalInput").ap(),
        "cold": nc.dram_tensor("cold", (128, COLDW), F32, kind="ExternalInput").ap(),
    }
    out = nc.dram_tensor("partials", (128, NPART), F32, kind="ExternalOutput").ap()

    with tile.TileContext(nc) as tc:
        emit(tc, out, ins)
    nc.compile()
    _CACHE["nc"] = nc
    return nc


def make_rec(inputs, lo, hi):
    """Per-cell records [16800, 64]: (obj, reg0..3, cls0..29, 0-pad).

    Pure relayout - all arithmetic happens on device. Row of cell (s,b,y,x)
    is REC_BASE[s] + b*HW_s + y*W_s + x.
    """
    rec = np.zeros((NREC, RECW), np.float32)
    for s, (h, w) in enumerate(SCALES):
        hw = h * w
        r0 = REC_BASE[s]
        n = B_SH * hw
        rec[r0 : r0 + n, 0] = np.asarray(inputs[f"obj_p{s}"][lo:hi]).reshape(n)
        rec[r0 : r0 + n, 1:5] = (
            np.asarray(inputs[f"reg_p{s}"][lo:hi])
            .reshape(B_SH, 4, hw).transpose(0, 2, 1).reshape(n, 4)
        )
        rec[r0 : r0 + n, 5 : 5 + C] = (
            np.asarray(inputs[f"cls_p{s}"][lo:hi])
            .reshape(B_SH, C, hw).transpose(0, 2, 1).reshape(n, C)
        )
    return rec


def make_hot(inputs, lo, hi, kc):
    hot = np.empty((128, HOTW), np.float32)
    hot[:, 0:4] = np.asarray(inputs["boxes"][lo:hi]).reshape(128, 4)
    hot[:, 4] = np.asarray(inputs["labels"][lo:hi]).reshape(128).astype(np.float32)
    hot[:, 5:20] = kc
    return hot


def make_cold(inputs, lo, hi, iota, ident, utri):
    cold = np.empty((128, COLDW), np.float32)
    cold[:, 0:30] = iota
    cold[:, 30:158] = ident
    cold[:, 158:286] = utri
    c0 = 286
    for s, ncol in enumerate(OBJ_COLS):
        flat = np.full(128 * ncol, OBJ_PAD, np.float32)
        v = np.asarray(inputs[f"obj_p{s}"][lo:hi]).reshape(-1)
        flat[: v.size] = v
        cold[:, c0 : c0 + ncol] = flat.reshape(128, ncol)
        c0 += ncol
    return cold


def combine_partials(parts):
    """parts: [n_cores, 128, 18] -> final [4] losses."""
    tot = np.asarray(parts, np.float64).sum(axis=(0, 1))
    cls_sum = reg_sum = obj_sum = 0.0
    for s, (h, w) in enumerate(SCALES):
        b = 6 * s
        lse, val, sl1, obj, sp, npos = tot[b : b + 6]
        npos = max(npos, 1.0)
        cls_sum += (lse - val) / npos * CLS_W
        reg_sum += sl1 / npos * REG_W
        obj_sum += (sp - obj) / (B_TOT * h * w) * OBJ_W
    cls_sum /= len(SCALES)
    reg_sum /= len(SCALES)
    obj_sum /= len(SCALES)
    total = cls_sum + reg_sum + obj_sum
    return np.array([total, cls_sum, reg_sum, obj_sum], np.float32)


TRACE = False
LAST_RESULT = None


def kernel(**inputs):
    global LAST_RESULT
    nc = _build()
    kc, iota, ident, utri = _host_consts()
    in_maps = []
    for c in range(N_CORES):
        lo, hi = c * B_SH, (c + 1) * B_SH
        in_maps.append({
            "rec": make_rec(inputs, lo, hi),
            "hot": make_hot(inputs, lo, hi, kc),
            "cold": make_cold(inputs, lo, hi, iota, ident, utri),
        })
    res = run_bass_kernel_spmd(
        nc, in_maps, core_ids=list(range(N_CORES)), trace=TRACE
    )
    LAST_RESULT = res
    parts = np.stack([np.asarray(r["partials"]) for r in res.results])
    return combine_partials(parts)
